# revision 2
# baseline (speedup 1.0000x reference)
"""Trainium2 Bass kernel for nn_MultiHeadAttention (B=2, L=2048, D=2048, H=16, RoPE, causal).

Sharding (8 cores): core c -> batch b = c//4, head-group g = c%4 (heads 4g..4g+3).
Each core computes its 4 heads' attention fully, plus a partial O-projection
(its 512 columns of the 2048-dim contraction); host sums the 4 partials per batch.

Device-side layouts are all "transposed" so the PE array never needs an
on-chip transpose:
  QT/KT  (d=128, l)   from  lhsT=wT chunk,  rhs=xT chunk
  V      (s, d)        from  lhsT=xT chunk,  rhs=wvT chunk
  scoresT(s, l)        from  lhsT=KT slice,  rhs=QT
  avT    (d, l)        from  lhsT=V slice,   rhs=P (=exp scoresT)
  out    (l, e)        from  lhsT=avT slice, rhs=woT
RoPE is applied in (d, l) layout: the W rows are pre-permuted on host so the
even/odd interleave becomes first-half/second-half of the 128 partitions.
Softmax runs without max-subtraction (scores are O(1) here); row sums come
from an M=1 ones-matmul; normalization uses gpsimd partition_broadcast of the
reciprocals plus a DVE multiply. All matmuls run in float32r (single-pass
fp32, ~1.7e-4 rel err measured on HW). Above-diagonal score blocks are
skipped entirely (outputs are donation-zeroed).
"""

import os
import sys
import types

import numpy as np

B, L, D, H = 2, 2048, 2048, 16
HD = D // H            # 128
N_CORES = 8
NHC = 4                # heads per core
NK = D // 128          # 16 k-chunks
NJ = L // 512          # 4 super-chunks of 512
SCALE = 1.0 / float(np.sqrt(HD))
NEG = -1.0e9

_PERM = np.concatenate([np.arange(0, HD, 2), np.arange(1, HD, 2)])  # even feats then odd

_prog = None  # cached compiled program


def _ensure_paths():
    for p in ("/opt/trn_rl_repo",):
        if os.path.isdir(p) and p not in sys.path:
            sys.path.insert(0, p)


def _install_ntff_hook():
    """Register the axon NTFF profile hook (antenv.axon_hooks is missing in the
    image) so run_bass_kernel_spmd(trace=True) can report HW exec time."""
    import antenv  # noqa: F401

    if "antenv.axon_hooks" not in sys.modules:
        mod = types.ModuleType("antenv.axon_hooks")
        mod._hook = None
        mod.set_axon_ntff_profile_hook = lambda h: setattr(mod, "_hook", h)
        mod.get_axon_ntff_profile_hook = lambda: mod._hook
        sys.modules["antenv.axon_hooks"] = mod
        antenv.axon_hooks = mod
    try:
        from trn_agent_boot.trn_boot import _ntff_profile_via_ctypes

        sys.modules["antenv.axon_hooks"].set_axon_ntff_profile_hook(
            _ntff_profile_via_ctypes("/opt/axon/libaxon_pjrt.so")
        )
    except Exception:
        pass
    import concourse.bass_utils as bu

    bu.upload_artifacts = lambda tmpdir: f"local:{tmpdir}"


def _rope_tables():
    inv_freq = 1.0 / (10000.0 ** (np.arange(0, HD, 2, dtype=np.float64) / HD))  # (64,)
    t = np.arange(L, dtype=np.float64)
    freqs = inv_freq[:, None] * t[None, :]           # (64, L)
    return np.cos(freqs).astype(np.float32), np.sin(freqs).astype(np.float32)


def _mask_tiles(attn_mask):
    """maskfull[dq] (128, 512): additive mask for a diagonal-containing scoresT
    tile (s part, l free); l sub-block q: keep q>dq, diagonal q==dq, mask q<dq."""
    blk = np.asarray(attn_mask[0, 0, :128, :128], dtype=np.float32).T  # [s, l]
    mf = np.zeros((4, 128, 512), dtype=np.float32)
    for dq in range(4):
        for q in range(4):
            if q == dq:
                mf[dq][:, q * 128:(q + 1) * 128] = blk
            elif q < dq:
                mf[dq][:, q * 128:(q + 1) * 128] = NEG
    return mf


def _host_prep(x_q, x_kv, attn_mask, w_q, w_k, w_v, w_o):
    """Build per-core input maps."""
    cosT, sinT = _rope_tables()
    maskf = _mask_tiles(attn_mask)
    xqT = [np.ascontiguousarray(np.asarray(x_q[b], dtype=np.float32).T) for b in range(B)]
    xkvT = [np.ascontiguousarray(np.asarray(x_kv[b], dtype=np.float32).T) for b in range(B)]

    def wT_perm(w, g):
        blk = np.asarray(w[g * 512:(g + 1) * 512], dtype=np.float32).reshape(4, HD, D)
        blk = blk[:, _PERM, :].reshape(512, D)
        return np.ascontiguousarray(blk.T)            # (2048, 512)

    in_maps = []
    for c in range(N_CORES):
        b, g = c // 4, c % 4
        in_maps.append({
            "xqT": xqT[b],
            "xkvT": xkvT[b],
            "wqT": wT_perm(w_q, g),
            "wkT": wT_perm(w_k, g),
            "wvT": np.ascontiguousarray(np.asarray(w_v[g * 512:(g + 1) * 512], dtype=np.float32).T),
            "woT": np.ascontiguousarray(np.asarray(w_o[:, g * 512:(g + 1) * 512], dtype=np.float32).T),
            "cosT": cosT,
            "sinT": sinT,
            "maskf": maskf,
        })
    return in_maps


def _build_program():
    _ensure_paths()
    import concourse.bacc as bacc
    import concourse.mybir as mybir
    from concourse import tile

    f32, f32r = mybir.dt.float32, mybir.dt.float32r
    Exp = mybir.ActivationFunctionType.Exp

    nc = bacc.Bacc("TRN2", target_bir_lowering=False, debug=False, num_devices=N_CORES)

    xqT = nc.dram_tensor("xqT", [D, L], f32, kind="ExternalInput")
    xkvT = nc.dram_tensor("xkvT", [D, L], f32, kind="ExternalInput")
    wqT = nc.dram_tensor("wqT", [D, 512], f32, kind="ExternalInput")
    wkT = nc.dram_tensor("wkT", [D, 512], f32, kind="ExternalInput")
    wvT = nc.dram_tensor("wvT", [D, 512], f32, kind="ExternalInput")
    woT = nc.dram_tensor("woT", [512, D], f32, kind="ExternalInput")
    cosT = nc.dram_tensor("cosT", [64, L], f32, kind="ExternalInput")
    sinT = nc.dram_tensor("sinT", [64, L], f32, kind="ExternalInput")
    maskf = nc.dram_tensor("maskf", [4, 128, 512], f32, kind="ExternalInput")

    attnT = nc.dram_tensor("attnT", [NHC, L, L], f32, kind="ExternalOutput")
    outp = nc.dram_tensor("outp", [L, D], f32, kind="ExternalOutput")

    with tile.TileContext(nc) as tc:
        with tc.tile_pool(name="persist", bufs=1) as pp:
            # persistent fp32r tiles: Q/K (d,l) per (h,j); V (s,d) per s-chunk
            qT = [[pp.tile([128, 512], f32r, tag=f"q{h}_{j}", name=f"q{h}_{j}") for j in range(NJ)] for h in range(NHC)]
            kT = [[pp.tile([128, 512], f32r, tag=f"k{h}_{j}", name=f"k{h}_{j}") for j in range(NJ)] for h in range(NHC)]
            vS = [pp.tile([128, 512], f32r, tag=f"v{sc}", name=f"v{sc}") for sc in range(16)]
            ones = pp.tile([128, 1], f32, tag="ones", name="ones")
            nc.vector.memset(ones[:], 1.0)
            onesr = pp.tile([128, 1], f32r, tag="onesr", name="onesr")
            nc.scalar.copy(out=onesr[:], in_=ones[:])

            def load_trig(pool, j, tagsuf=""):
                cs = pool.tile([64, 512], f32, tag="cs" + tagsuf, bufs=2, name="cs")
                sn = pool.tile([64, 512], f32, tag="sn" + tagsuf, bufs=2, name="sn")
                nc.sync.dma_start(out=cs[:], in_=cosT[:, j * 512:(j + 1) * 512])
                nc.sync.dma_start(out=sn[:], in_=sinT[:, j * 512:(j + 1) * 512])
                return cs, sn

            def rope(ps, dst, cs, sn, tmp_pool, tagsuf=""):
                """dst[0:64] = ps[0:64]*cos - ps[64:]*sin ; dst[64:] = ps[0:64]*sin + ps[64:]*cos"""
                pe_, po_ = ps[0:64, :], ps[64:128, :]
                t0 = tmp_pool.tile([64, 512], f32, tag="t0" + tagsuf, bufs=2, name="t0")
                t1 = tmp_pool.tile([64, 512], f32, tag="t1" + tagsuf, bufs=2, name="t1")
                nc.vector.tensor_mul(out=t0[:], in0=pe_, in1=cs[:])
                nc.vector.tensor_mul(out=t1[:], in0=po_, in1=sn[:])
                nc.vector.tensor_sub(out=dst[0:64, :], in0=t0[:], in1=t1[:])
                t2 = tmp_pool.tile([64, 512], f32, tag="t0" + tagsuf, bufs=2, name="t2")
                t3 = tmp_pool.tile([64, 512], f32, tag="t1" + tagsuf, bufs=2, name="t3")
                nc.vector.tensor_mul(out=t2[:], in0=pe_, in1=sn[:])
                nc.vector.tensor_mul(out=t3[:], in0=po_, in1=cs[:])
                nc.vector.tensor_add(out=dst[64:128, :], in0=t2[:], in1=t3[:])

            def stream_x(pool, src, k, j, tagsuf):
                xf = pool.tile([128, 512], f32, tag="xf" + tagsuf, bufs=3, name="xf")
                nc.sync.dma_start(out=xf[:], in_=src[k * 128:(k + 1) * 128, j * 512:(j + 1) * 512])
                xr = pool.tile([128, 512], f32r, tag="xr" + tagsuf, bufs=3, name="xr")
                nc.scalar.copy(out=xr[:], in_=xf[:])
                return xr

            def load_weights(pool, src, prefix):
                ws = []
                for k in range(NK):
                    wf = pool.tile([128, 512], f32, tag="wf" + prefix, bufs=2, name="wf")
                    nc.sync.dma_start(out=wf[:], in_=src[k * 128:(k + 1) * 128, :])
                    wr = pool.tile([128, 512], f32r, tag=f"{prefix}{k}", name=f"{prefix}{k}")
                    nc.scalar.copy(out=wr[:], in_=wf[:])
                    ws.append(wr)
                return ws

            # ---- phase 1: Q projection + RoPE (psq bufs=2 -> 8 banks, rope reads PSUM) ----
            with tc.tile_pool(name="p1", bufs=1) as p1, \
                 tc.tile_pool(name="ps1", bufs=1, space="PSUM") as ps1:
                wq = load_weights(p1, wqT, "wq")
                for j in range(NJ):
                    cs, sn = load_trig(p1, j)
                    psq = [ps1.tile([128, 512], f32, tag=f"psq{h}", bufs=2, name=f"psq{h}") for h in range(NHC)]
                    for k in range(NK):
                        xr = stream_x(p1, xqT, k, j, "")
                        for h in range(NHC):
                            nc.tensor.matmul(out=psq[h][:], lhsT=wq[k][:, h * 128:(h + 1) * 128],
                                             rhs=xr[:], start=(k == 0), stop=(k == NK - 1))
                    for h in range(NHC):
                        rope(psq[h], qT[h][j], cs, sn, p1)

            # ---- phase 2: K projection + RoPE ----
            with tc.tile_pool(name="p2", bufs=1) as p2, \
                 tc.tile_pool(name="ps2", bufs=1, space="PSUM") as ps2:
                wk_ = load_weights(p2, wkT, "wk")
                for j in range(NJ):
                    cs, sn = load_trig(p2, j, "k")
                    psk = [ps2.tile([128, 512], f32, tag=f"psk{h}", bufs=2, name=f"psk{h}") for h in range(NHC)]
                    for k in range(NK):
                        xr = stream_x(p2, xkvT, k, j, "k")
                        for h in range(NHC):
                            nc.tensor.matmul(out=psk[h][:], lhsT=wk_[k][:, h * 128:(h + 1) * 128],
                                             rhs=xr[:], start=(k == 0), stop=(k == NK - 1))
                    for h in range(NHC):
                        rope(psk[h], kT[h][j], cs, sn, p2, "k")

            # ---- phase 3: V projection (natural layout), second xkvT stream ----
            with tc.tile_pool(name="p3", bufs=1) as p3, \
                 tc.tile_pool(name="ps3", bufs=1, space="PSUM") as ps3:
                wv_ = load_weights(p3, wvT, "wv")
                for j in range(NJ):
                    psv = [ps3.tile([128, 512], f32, tag=f"psv{q}", bufs=2, name=f"psv{q}") for q in range(4)]
                    for k in range(NK):
                        xr = stream_x(p3, xkvT, k, j, "v")
                        for q in range(4):
                            nc.tensor.matmul(out=psv[q][:], lhsT=xr[:, q * 128:(q + 1) * 128],
                                             rhs=wv_[k][:], start=(k == 0), stop=(k == NK - 1))
                    for q in range(4):
                        nc.scalar.copy(out=vS[j * 4 + q][:], in_=psv[q][:])

            # ---- phases 4+5 share the avT pool ----
            with tc.tile_pool(name="pav", bufs=1) as pav:
                avT = [[pav.tile([128, 512], f32r, tag=f"av{h}_{j}", name=f"av{h}_{j}") for j in range(NJ)] for h in range(NHC)]

                # ---- phase 4: attention ----
                with tc.tile_pool(name="p4", bufs=1) as p4, \
                     tc.tile_pool(name="ps4", bufs=1, space="PSUM") as ps4:
                    mky = [p4.tile([128, 512], f32, tag=f"mk{dq}", name=f"mk{dq}") for dq in range(4)]
                    for dq in range(4):
                        nc.sync.dma_start(out=mky[dq][:], in_=maskf[dq])
                    for h in range(NHC):
                        for j in range(NJ):
                            n_s = 4 * (j + 1)
                            plist = []
                            pssum = ps4.tile([1, 512], f32, tag="pssum", bufs=2, name="pssum")
                            psav = ps4.tile([128, 512], f32, tag="psav", bufs=2, name="psav")
                            for sc in range(n_s):
                                pss = ps4.tile([128, 512], f32, tag="pss", bufs=3, name="pss")
                                nc.tensor.matmul(out=pss[:], lhsT=kT[h][sc // 4][:, (sc % 4) * 128:(sc % 4 + 1) * 128],
                                                 rhs=qT[h][j][:], start=True, stop=True)
                                p = p4.tile([128, 512], f32r, tag="p", bufs=16, name="p")
                                dq = sc - 4 * j
                                if dq >= 0:
                                    nc.vector.tensor_add(out=pss[:], in0=pss[:], in1=mky[dq][:])
                                nc.scalar.activation(out=p[:], in_=pss[:], func=Exp, scale=SCALE)
                                nc.tensor.matmul(out=pssum[:], lhsT=onesr[:], rhs=p[:],
                                                 start=(sc == 0), stop=(sc == n_s - 1))
                                nc.tensor.matmul(out=psav[:], lhsT=vS[sc][:, h * 128:(h + 1) * 128],
                                                 rhs=p[:], start=(sc == 0), stop=(sc == n_s - 1))
                                plist.append(p)
                            inv = p4.tile([1, 512], f32, tag="inv", bufs=2, name="inv")
                            nc.vector.reciprocal(out=inv[:], in_=pssum[:])
                            invb = p4.tile([128, 512], f32, tag="invb", bufs=2, name="invb")
                            nc.gpsimd.partition_broadcast(invb[:], inv[:])
                            for sc, p in enumerate(plist):
                                pn = p4.tile([128, 512], f32, tag="pn", bufs=4, name="pn")
                                nc.vector.tensor_mul(out=pn[:], in0=p[:], in1=invb[:])
                                nc.sync.dma_start(out=attnT[h, sc * 128:(sc + 1) * 128, j * 512:(j + 1) * 512],
                                                  in_=pn[:])
                            nc.vector.tensor_mul(out=avT[h][j][:], in0=psav[:], in1=invb[:])

                # ---- phase 5: O projection (partial over this core's 512 dims) ----
                with tc.tile_pool(name="p5", bufs=1) as p5, \
                     tc.tile_pool(name="ps5", bufs=1, space="PSUM") as ps5:
                    wo = []
                    for d4 in range(4):
                        wf = p5.tile([128, D], f32, tag="wof", bufs=2, name="wf")
                        nc.sync.dma_start(out=wf[:], in_=woT[d4 * 128:(d4 + 1) * 128, :])
                        wr = p5.tile([128, D], f32r, tag=f"wo{d4}", name=f"wo{d4}")
                        nc.scalar.copy(out=wr[:], in_=wf[:])
                        wo.append(wr)
                    for lc in range(16):
                        for e in range(4):
                            pso = ps5.tile([128, 512], f32, tag="pso", bufs=4, name="pso")
                            for d4 in range(4):
                                nc.tensor.matmul(out=pso[:], lhsT=avT[d4][lc // 4][:, (lc % 4) * 128:(lc % 4 + 1) * 128],
                                                 rhs=wo[d4][:, e * 512:(e + 1) * 512],
                                                 start=(d4 == 0), stop=(d4 == 3))
                            ot = p5.tile([128, 512], f32, tag="ot", bufs=4, name="ot")
                            nc.scalar.copy(out=ot[:], in_=pso[:])
                            nc.sync.dma_start(out=outp[lc * 128:(lc + 1) * 128, e * 512:(e + 1) * 512], in_=ot[:])

    nc.compile()
    return nc


def _get_program():
    global _prog
    if _prog is None:
        _prog = _build_program()
    return _prog


def kernel(x_q, x_kv, attn_mask, w_q, w_k, w_v, w_o):
    _ensure_paths()
    _install_ntff_hook()
    from concourse.bass_utils import run_bass_kernel_spmd

    nc = _get_program()
    in_maps = _host_prep(x_q, x_kv, attn_mask, w_q, w_k, w_v, w_o)
    trace = bool(int(os.environ.get("BASS_KERNEL_TRACE", "0")))
    res = run_bass_kernel_spmd(nc, in_maps, list(range(N_CORES)), trace=trace)
    if trace:
        kernel.last_exec_time_ns = res.exec_time_ns

    out = np.zeros((B, L, D), dtype=np.float32)
    attn = np.empty((B, H, L, L), dtype=np.float32)
    for c in range(N_CORES):
        b, g = c // 4, c % 4
        r = res.results[c]
        out[b] += r["outp"]
        at = r["attnT"]                       # (4, S, L)
        for h in range(NHC):
            attn[b, 4 * g + h] = at[h].T
    return out, attn


kernel.last_exec_time_ns = None


def numpy_mirror(x_q, x_kv, attn_mask, w_q, w_k, w_v, w_o):
    """Pure-numpy mirror of the per-core dataflow (fp32 math, no fp32r
    rounding) to validate sharding/layout/permutation logic."""
    in_maps = _host_prep(x_q, x_kv, attn_mask, w_q, w_k, w_v, w_o)
    out = np.zeros((B, L, D), dtype=np.float32)
    attn = np.zeros((B, H, L, L), dtype=np.float32)
    for c in range(N_CORES):
        b, g = c // 4, c % 4
        m = in_maps[c]
        cs, sn = m["cosT"], m["sinT"]
        for h in range(NHC):
            qTh = np.zeros((128, L), dtype=np.float32)
            kTh = np.zeros((128, L), dtype=np.float32)
            for j in range(NJ):
                sl = slice(j * 512, (j + 1) * 512)
                psq = m["wqT"][:, h * 128:(h + 1) * 128].T @ m["xqT"][:, sl]
                psk = m["wkT"][:, h * 128:(h + 1) * 128].T @ m["xkvT"][:, sl]
                for ps, dst in ((psq, qTh), (psk, kTh)):
                    dst[0:64, sl] = ps[0:64] * cs[:, sl] - ps[64:128] * sn[:, sl]
                    dst[64:128, sl] = ps[0:64] * sn[:, sl] + ps[64:128] * cs[:, sl]
            vh = (m["xkvT"].T @ m["wvT"][:, h * 128:(h + 1) * 128])    # (s, 128)
            for j in range(NJ):
                n_s = 4 * (j + 1)
                sl = slice(j * 512, (j + 1) * 512)
                P = np.zeros((n_s * 128, 512), dtype=np.float32)
                for sc in range(n_s):
                    pss = kTh[:, sc * 128:(sc + 1) * 128].T @ qTh[:, sl]
                    dq = sc - 4 * j
                    if dq >= 0:
                        pss = pss + m["maskf"][dq]
                    P[sc * 128:(sc + 1) * 128] = np.exp(SCALE * pss)
                inv = 1.0 / P.sum(axis=0, keepdims=True)
                Pn = P * inv
                attn[b, 4 * g + h][sl, :n_s * 128] = Pn.T
                avn = vh[:n_s * 128].T @ Pn                              # (128 d, 512 l)
                out[b][sl] += avn.T @ m["woT"][h * 128:(h + 1) * 128, :]
    return out, attn
